# revision 24
# baseline (speedup 1.0000x reference)
"""Trainium2 Bass kernel for the FC-SNN (LIF hidden layer + LI readout).

Structure (per core, batch-sharded B=512 -> 64):
  host:   i1[t] is a spike-independent linear filter of x, so it is folded into
          a prefilter of x (exact reparameterization); layout/transpose.
  device: S = (0.1*xfilt) @ w1T  (big matmul, time-parallel)
          v-scan over t (threshold + reset, the only sequential part)
          oc = z @ w_outT        (spike readout matmul)
          LI readout (vo/io scans) + max over t
  host:   gather [10,64] per core -> [512,10].

Engine assignment (r1 mode):
  PE:   fc1 matmuls (fp32r, 1 pass) + fc_out matmuls (bf16)
  ACT:  PSUM->SBUF copies of S
  DVE:  the serial v-chain (reset-mask + decay-add), on-engine back-to-back
  Pool: spike compare (z, bf16) + LI readout (reads oc straight from PSUM)
  x DMAs ride the gpsimd queue, w DMAs the sync queue, so the first x
  group isn't stuck behind the 12.9MB weight load.

Groups taper [8]*7+[4,4] so the pipeline drain (scan+fc_out+readout of
the last group, which nothing can overlap) is short.
"""

import sys

if "/opt/trn_rl_repo" not in sys.path:
    sys.path.insert(0, "/opt/trn_rl_repo")

from contextlib import ExitStack

import numpy as np

# problem dims (hardcoded per contract)
T, B, C, Hh, Ww = 64, 512, 1, 28, 28
IN, HID, OUT = 784, 2048, 10
NCORES = 8
BL = B // NCORES            # 64 batch rows per core
TB = T * BL                 # 4096 matmul columns per core
KC = 7                      # contraction chunks: 784 = 7*112 (no padding)
KP = IN // KC               # 112 rows per chunk
MC = HID // 128             # 16 hidden chunks
TG = 8                      # max time steps per pipeline group
NVD = 4                     # vd state ring buffers (slot t+2 doubles as scratch)

# "bf3": 3-pass bf16 split fc1 (near-exact), "r1": 1-pass fp32r (fast, ~6e-3 rel)
MODE = "r1"
TRACE = False

_CACHE = {}
LAST_RESULT = None


def _round_mant(a, mbits):
    """Round fp32 mantissa to mbits (round-to-nearest, matching fp32r pre-round)."""
    ai = np.ascontiguousarray(a, np.float32).view(np.uint32).astype(np.uint64)
    half = np.uint64(1) << np.uint64(22 - mbits)
    mask = np.uint64(0xFFFFFFFF) << np.uint64(23 - mbits)
    return ((ai + half) & mask).astype(np.uint32).view(np.float32)


def _to_bf16(a):
    import ml_dtypes
    return np.ascontiguousarray(a).astype(ml_dtypes.bfloat16)


def _layout_x(arr):
    """[TB, IN] -> [KP, KC, TB] (p-major k-chunks, no padding)."""
    return np.ascontiguousarray(
        arr.T.reshape(KC, KP, TB).transpose(1, 0, 2)
    )


def _layout_w(arr):
    """[IN, HID] -> [KP, KC, HID]."""
    return np.ascontiguousarray(
        arr.reshape(KC, KP, HID).transpose(1, 0, 2)
    )


def _layout_w2(arr):
    """[HID, OUT] -> [128, MC*OUT]."""
    return np.ascontiguousarray(
        arr.reshape(MC, 128, OUT).transpose(1, 0, 2).reshape(128, MC * OUT)
    )


def _group_sizes(tgmax):
    tgs = []
    rem = T
    while rem > tgmax:
        tgs.append(tgmax)
        rem -= tgmax
    # taper the tail so the drain is short (fp32r needs >=4 steps/group)
    if rem > 4:
        h = rem // 2
        tgs += [h, rem - h]
    else:
        tgs.append(rem)
    return tgs


def _build_nc(mode, tg=TG, repeat=1, level=5):
    import concourse.bacc as bacc
    import concourse.mybir as mybir
    import concourse.tile as tile

    f32 = mybir.dt.float32
    f32r = mybir.dt.float32r
    bf16 = mybir.dt.bfloat16
    Alu = mybir.AluOpType
    Act = mybir.ActivationFunctionType

    nc = bacc.Bacc("TRN2", debug=False)
    tgs = _group_sizes(tg)
    ng = len(tgs)
    t0s = [sum(tgs[:i]) for i in range(ng)]
    nvd = NVD
    xbufs = 2 if tg > 4 else 3

    if mode == "r1":
        xdt = wdt = f32r
        nxs, nws = 1, 1
        passes = [(0, 0)]
    else:
        xdt = wdt = bf16
        nxs, nws = 2, 2
        passes = [(0, 0), (0, 1), (1, 0)]

    xps = [
        nc.declare_dram_parameter(f"x{i}", [KP, KC, TB], xdt, isOutput=False)
        for i in range(nxs)
    ]
    wps = [
        nc.declare_dram_parameter(f"w{i}", [KP, KC, HID], wdt, isOutput=False)
        for i in range(nws)
    ]
    w2ps = [
        nc.declare_dram_parameter("w20", [128, MC * OUT], bf16, isOutput=False)
    ]
    vmax_p = nc.declare_dram_parameter("vmax", [OUT, BL], f32, isOutput=True)

    with tile.TileContext(nc) as tc, ExitStack() as ctx:
        const = ctx.enter_context(tc.tile_pool(name="const", bufs=1))
        xpool = ctx.enter_context(tc.tile_pool(name="x", bufs=xbufs))
        swpool = ctx.enter_context(tc.tile_pool(name="sw", bufs=2))
        sgpool = ctx.enter_context(tc.tile_pool(name="sg", bufs=2))
        psS = ctx.enter_context(tc.tile_pool(name="psS", bufs=6, space="PSUM"))
        psO = ctx.enter_context(tc.tile_pool(name="psO", bufs=2, space="PSUM"))
        ocpool = ctx.enter_context(tc.tile_pool(name="oc", bufs=2))
        spool = ctx.enter_context(tc.tile_pool(name="sgn", bufs=2))

        # persistent tiles (w DMAs are emitted inside whole_body, per-m
        # chunks on the sync queue, after the first x load)
        wts = [
            const.tile([KP, KC * HID], wdt, tag=f"w{i}", name=f"w{i}")
            for i in range(nws)
        ]
        w2t = const.tile([128, MC * OUT], bf16, tag="w20", name="w20")
        vd = [const.tile([128, MC * BL], f32, tag=f"vd{i}", name=f"vd{i}") for i in range(nvd)]
        nhalf = const.tile([128, 1], f32, tag="nhalf", name="nhalf")
        js = const.tile([OUT, BL], f32, tag="js", name="js")
        vo = const.tile([OUT, BL], f32, tag="vo", name="vo")
        vmax_t = const.tile([OUT, BL], f32, tag="vmax", name="vmax")

        def dma_w():
            for i in range(nws):
                wap = wts[i][:].rearrange("p (k h) -> p k h", k=KC)
                for m in range(MC):
                    nc.sync.dma_start(
                        wap[:, :, m * 128:(m + 1) * 128],
                        wps[i][:, :, m * 128:(m + 1) * 128],
                    )
            nc.sync.dma_start(w2t[:], w2ps[0][:, :])

        def dma_x(g):
            c0, ncol = t0s[g] * BL, tgs[g] * BL
            tiles = []
            for si in range(nxs):
                xt = xpool.tile([KP, KC * tg * BL], xdt, tag=f"x{si}", name=f"x{si}")
                xv = xt[:, :KC * ncol].rearrange("p (k c) -> p k c", k=KC)
                if g <= 1:
                    # per-k chunks, round-robin across idle queues, so the
                    # first matmuls aren't gated on one serial SWDGE stream
                    qs = [nc.gpsimd, nc.scalar]
                    for k in range(KC):
                        qs[k % len(qs)].dma_start(
                            xv[:, k:k + 1, :], xps[si][:, k:k + 1, c0:c0 + ncol]
                        )
                else:
                    nc.gpsimd.dma_start(xv, xps[si][:, :, c0:c0 + ncol])
                tiles.append(xt)
            return tiles

        def fc1(g, xts):
            tgl = tgs[g]
            ncol = tgl * BL
            sw = swpool.tile([128, tg * MC * BL], f32, tag="swin", name="swin")
            for m in range(MC):
                ps = psS.tile([128, tg * BL], f32, tag="psS", name="psS")
                idx, last = 0, len(passes) * KC - 1
                for (xi, wi) in passes:
                    wap = wts[wi][:].rearrange("p (k h) -> p k h", k=KC)
                    xap = xts[xi][:, :KC * ncol].rearrange(
                        "p (k c) -> p k c", k=KC
                    )
                    for k in range(KC):
                        lhsT = wap[:, k, m * 128:(m + 1) * 128]
                        rhs = xap[:, k, :]
                        nc.tensor.matmul(
                            ps[:, :ncol], lhsT, rhs,
                            start=(idx == 0), stop=(idx == last)
                        )
                        idx += 1
                dst = sw[:, :tgl * MC * BL].rearrange(
                    "p (t m b) -> p t m b", t=tgl, m=MC
                )[:, :, m, :]
                nc.scalar.copy(
                    dst, ps[:, :ncol].rearrange("p (t b) -> p t b", t=tgl)
                )
            return sw

        def ro_step(oc, tloc):
            # one LI-readout step (DVE; GPSIMD has no TensorScalarPtr on HW)
            nc.vector.scalar_tensor_tensor(
                vo[:], vo[:], 0.9, js[:], op0=Alu.mult, op1=Alu.add
            )
            nc.vector.tensor_tensor(vmax_t[:], vmax_t[:], vo[:], op=Alu.max)
            nc.vector.scalar_tensor_tensor(
                js[:], js[:], 0.8, oc[:, tloc * BL:(tloc + 1) * BL],
                op0=Alu.mult, op1=Alu.add,
            )

        def scan(g, sw, ro):
            # sg is (m t b) so fc_out's rhs slices are contiguous 2-D
            tgl = tgs[g]
            sg = sgpool.tile([128, tg * MC * BL], bf16, tag="sgn", name="sgn")
            sgap = sg[:, :tgl * MC * BL].rearrange(
                "p (m t b) -> p m t b", m=MC, t=tgl
            )
            w = MC * BL
            if level < 2:
                return sg
            for tloc in range(tgl):
                t = t0s[g] + tloc
                a = vd[t % nvd]
                b = vd[(t + 1) % nvd]
                vr = vd[(t + 2) % nvd]
                # DVE: the serial v-chain, back-to-back on one engine
                nc.vector.scalar_tensor_tensor(
                    vr[:], a[:], 0.5, a[:], op0=Alu.is_le, op1=Alu.mult
                )
                nc.vector.scalar_tensor_tensor(
                    b[:], vr[:], 0.9, sw[:, tloc * w:(tloc + 1) * w],
                    op0=Alu.mult, op1=Alu.add,
                )
                if level >= 3:
                    # spike values, off the serial chain: ACT computes
                    # sign(v-0.5) in bf16, DVE finishes z=max(s,0) at
                    # 2-byte (2x) rate
                    st = spool.tile([128, MC * BL], bf16, tag="st", name="st")
                    nc.scalar.activation(st[:], a[:], Act.Sign, bias=nhalf[:])
                    nc.vector.tensor_scalar(
                        sgap[:, :, tloc, :], st[:].rearrange(
                            "p (m b) -> p m b", m=MC
                        ), 0.0, None, op0=Alu.max
                    )
                if ro and level >= 5:
                    ro_step(*ro.pop(0))
                    if len(ro) > tgl - 1 - tloc:  # backlog: drain 2 per step
                        ro_step(*ro.pop(0))
            return sg

        def fc_out(g, sg):
            # Pool can't read PSUM on real HW, so oc hops through SBUF (ACT)
            tgl = tgs[g]
            po = psO.tile([OUT, tg * BL], f32, tag="psO", name="psO")
            oc = ocpool.tile([OUT, tg * BL], f32, tag="ocs", name="ocs")
            if level < 4:
                return oc
            for m in range(MC):
                rhs = sg[:, m * tgl * BL:(m + 1) * tgl * BL]
                lhsT = w2t[:, m * OUT:(m + 1) * OUT]
                nc.tensor.matmul(
                    po[:, :tgl * BL], lhsT, rhs,
                    start=(m == 0), stop=(m == MC - 1)
                )
            nc.scalar.copy(oc[:, :tgl * BL], po[:, :tgl * BL])
            return oc

        def whole_body():
            # init state, then software-pipelined emission: fc1 leads the
            # scan by 1 group; fc_out of g-2 is emitted before scan(g-1),
            # whose Pool loop carries the interleaved readout of g-2.
            xts = dma_x(0)
            dma_w()
            nc.vector.memset(vd[0][:], 0.0)
            nc.vector.memset(js[:], 0.0)
            nc.vector.memset(vo[:], 0.0)
            nc.vector.memset(vmax_t[:], 0.0)
            nc.vector.memset(nhalf[:], -0.5)
            sws, sgs = {}, {}
            ro = []
            sws[0] = fc1(0, xts)
            for g in range(1, ng):
                xts = dma_x(g)
                sws[g] = fc1(g, xts)
                if g >= 2:
                    po = fc_out(g - 2, sgs.pop(g - 2))
                    ro.extend((po, i) for i in range(tgs[g - 2]))
                sgs[g - 1] = scan(g - 1, sws.pop(g - 1), ro)
            po = fc_out(ng - 2, sgs.pop(ng - 2))
            ro.extend((po, i) for i in range(tgs[ng - 2]))
            sgs[ng - 1] = scan(ng - 1, sws.pop(ng - 1), ro)
            while ro:  # leftovers from tapered groups, in order
                ro_step(*ro.pop(0))
            # last group's fc_out in 2-step column chunks, readout
            # interleaved, so the tail isn't one long serial chain
            tgl = tgs[ng - 1]
            sg = sgs.pop(ng - 1)
            po = psO.tile([OUT, tg * BL], f32, tag="psO", name="psO")
            oc = ocpool.tile([OUT, tg * BL], f32, tag="ocs", name="ocs")
            for tloc0 in range(0, tgl, 2):
                c0, c1 = tloc0 * BL, (tloc0 + 2) * BL
                if level >= 4:
                    for m in range(MC):
                        nc.tensor.matmul(
                            po[:, c0:c1],
                            w2t[:, m * OUT:(m + 1) * OUT],
                            sg[:, m * tgl * BL + c0:m * tgl * BL + c1],
                            start=(m == 0), stop=(m == MC - 1),
                        )
                    nc.scalar.copy(oc[:, c0:c1], po[:, c0:c1])
                if level >= 5:
                    ro_step(oc, tloc0)
                    ro_step(oc, tloc0 + 1)
            nc.sync.dma_start(vmax_p[:, :], vmax_t[:])

        if repeat > 1:
            with tc.For_i(0, repeat, 1):
                whole_body()
        else:
            whole_body()

    nc.compile()
    return nc


def _prep_inputs(x, w1, w_out, mode):
    x = np.ascontiguousarray(x, np.float32).reshape(T, B, IN)
    # i1[t] = 0.8*i1[t-1] + x[t] @ w1T  ==  prefilter(x)[t] @ w1T
    xf = np.empty_like(x)
    acc = np.zeros((B, IN), np.float32)
    e8 = np.float32(0.8)
    for t in range(T):
        acc = e8 * acc + x[t]
        xf[t] = acc
    xs = np.float32(0.1) * xf                       # S = xs @ w1T
    w1T = np.ascontiguousarray(w1, np.float32).T    # [IN, HID]

    if mode == "r1":
        xparts = [_round_mant(xs, 11)]
        wparts = [_layout_w(_round_mant(w1T, 11))]
    else:
        xh = _to_bf16(xs)
        xl = _to_bf16(xs - xh.astype(np.float32))
        xparts = [xh, xl]
        wh = _to_bf16(w1T)
        wl = _to_bf16(w1T - wh.astype(np.float32))
        wparts = [_layout_w(wh), _layout_w(wl)]

    w2 = np.float32(0.1) * np.ascontiguousarray(w_out, np.float32).T  # [HID,OUT]

    common = {
        "w20": _layout_w2(_to_bf16(w2)),
    }
    for i, wp in enumerate(wparts):
        common[f"w{i}"] = wp

    in_maps = []
    for c in range(NCORES):
        m = dict(common)
        for i, xp in enumerate(xparts):
            xc = xp[:, c * BL:(c + 1) * BL, :].reshape(TB, IN)
            m[f"x{i}"] = _layout_x(xc)
        in_maps.append(m)
    return in_maps


def kernel(x, w1, w_out):
    global LAST_RESULT
    from concourse.bass_utils import run_bass_kernel_spmd

    if MODE not in _CACHE:
        _CACHE[MODE] = _build_nc(MODE, tg=TG)
    nc = _CACHE[MODE]
    in_maps = _prep_inputs(np.asarray(x), np.asarray(w1), np.asarray(w_out), MODE)
    res = run_bass_kernel_spmd(nc, in_maps, list(range(NCORES)), trace=TRACE)
    LAST_RESULT = res
    out = np.empty((B, OUT), np.float32)
    for c in range(NCORES):
        out[c * BL:(c + 1) * BL, :] = np.asarray(res.results[c]["vmax"]).T
    return out


# revision 35
# speedup vs baseline: 47.7268x; 47.7268x over previous
"""Trainium2 Bass kernel for the FC-SNN (LIF hidden layer + LI readout).

Structure (per core, batch-sharded B=512 -> 64):
  host:   i1[t] is a spike-independent linear filter of x, so it is folded into
          a prefilter of x (exact reparameterization); layout/transpose.
  device: S = (0.1*xfilt) @ w1T  (big matmul, time-parallel)
          v-scan over t (threshold + reset, the only sequential part)
          oc = z @ w_outT        (spike readout matmul)
          LI readout (vo/io scans) + max over t
  host:   gather [10,64] per core -> [512,10].

Engine assignment (r1 mode):
  PE:   fc1 matmuls (fp32r, 1 pass) + fc_out matmuls (bf16)
  ACT:  PSUM->SBUF copies of S
  DVE:  the serial v-chain (reset-mask + decay-add), on-engine back-to-back
  Pool: spike compare (z, bf16) + LI readout (reads oc straight from PSUM)
  x DMAs ride the gpsimd queue, w DMAs the sync queue, so the first x
  group isn't stuck behind the 12.9MB weight load.

Groups taper [8]*7+[4,4] so the pipeline drain (scan+fc_out+readout of
the last group, which nothing can overlap) is short.
"""

import sys

if "/opt/trn_rl_repo" not in sys.path:
    sys.path.insert(0, "/opt/trn_rl_repo")

from contextlib import ExitStack

import numpy as np

# problem dims (hardcoded per contract)
T, B, C, Hh, Ww = 64, 512, 1, 28, 28
IN, HID, OUT = 784, 2048, 10
NCORES = 8
BL = B // NCORES            # 64 batch rows per core
TB = T * BL                 # 4096 matmul columns per core
KC = 7                      # contraction chunks: 784 = 7*112 (no padding)
KP = IN // KC               # 112 rows per chunk
MC = HID // 128             # 16 hidden chunks
TG = 8                      # max time steps per pipeline group
NVD = 4                     # vd state ring buffers (slot t+2 doubles as scratch)

# "bf3": 3-pass bf16 split fc1 (near-exact), "r1": 1-pass fp32r (fast, ~6e-3 rel)
MODE = "r1"
TRACE = False
# LI readout as one H-matrix matmul + max-reduce at the end (True) instead of
# 3 small DVE ops per timestep (False)
RO_HMAT = True
NCH = (OUT * BL) // 128     # 5 column chunks of the transposed oc

_CACHE = {}
LAST_RESULT = None


def _round_mant(a, mbits):
    """Round fp32 mantissa to mbits (round-to-nearest, matching fp32r pre-round)."""
    ai = np.ascontiguousarray(a, np.float32).view(np.uint32).astype(np.uint64)
    half = np.uint64(1) << np.uint64(22 - mbits)
    mask = np.uint64(0xFFFFFFFF) << np.uint64(23 - mbits)
    return ((ai + half) & mask).astype(np.uint32).view(np.float32)


def _to_bf16(a):
    import ml_dtypes
    return np.ascontiguousarray(a).astype(ml_dtypes.bfloat16)


def _layout_x(arr):
    """[TB, IN] -> [KP, KC, TB] (p-major k-chunks, no padding)."""
    return np.ascontiguousarray(
        arr.T.reshape(KC, KP, TB).transpose(1, 0, 2)
    )


def _layout_w(arr):
    """[IN, HID] -> [KP, KC, HID]."""
    return np.ascontiguousarray(
        arr.reshape(KC, KP, HID).transpose(1, 0, 2)
    )


def _layout_w2(arr):
    """[HID, OUT] -> [128, MC*OUT]."""
    return np.ascontiguousarray(
        arr.reshape(MC, 128, OUT).transpose(1, 0, 2).reshape(128, MC * OUT)
    )


def _group_sizes(tgmax):
    tgs = []
    rem = T
    while rem > tgmax:
        tgs.append(tgmax)
        rem -= tgmax
    # taper the tail so the drain is short (fp32r needs >=4 steps/group)
    if rem > 4:
        h = rem // 2
        tgs += [h, rem - h]
    else:
        tgs.append(rem)
    return tgs


def _build_nc(mode, tg=TG, repeat=1, level=5):
    import concourse.bacc as bacc
    import concourse.mybir as mybir
    import concourse.tile as tile

    f32 = mybir.dt.float32
    f32r = mybir.dt.float32r
    bf16 = mybir.dt.bfloat16
    Alu = mybir.AluOpType
    Act = mybir.ActivationFunctionType

    nc = bacc.Bacc("TRN2", debug=False)
    tgs = _group_sizes(tg)
    ng = len(tgs)
    t0s = [sum(tgs[:i]) for i in range(ng)]
    nvd = NVD
    xbufs = 2 if tg > 4 else 3

    if mode == "r1":
        xdt = wdt = f32r
        nxs, nws = 1, 1
        passes = [(0, 0)]
    else:
        xdt = wdt = bf16
        nxs, nws = 2, 2
        passes = [(0, 0), (0, 1), (1, 0)]

    xps = [
        nc.declare_dram_parameter(f"x{i}", [KP, KC, TB], xdt, isOutput=False)
        for i in range(nxs)
    ]
    wps = [
        nc.declare_dram_parameter(f"w{i}", [KP, KC, HID], wdt, isOutput=False)
        for i in range(nws)
    ]
    w2ps = [
        nc.declare_dram_parameter("w20", [128, MC * OUT], bf16, isOutput=False)
    ]
    if RO_HMAT:
        hs_p = nc.declare_dram_parameter("hs", [T, T], f32, isOutput=False)
        vmax_p = nc.declare_dram_parameter("vmax", [128, NCH], f32, isOutput=True)
    else:
        vmax_p = nc.declare_dram_parameter("vmax", [OUT, BL], f32, isOutput=True)

    with tile.TileContext(nc) as tc, ExitStack() as ctx:
        const = ctx.enter_context(tc.tile_pool(name="const", bufs=1))
        xpool = ctx.enter_context(tc.tile_pool(name="x", bufs=xbufs))
        swpool = ctx.enter_context(tc.tile_pool(name="sw", bufs=2))
        sgpool = ctx.enter_context(tc.tile_pool(name="sg", bufs=2))
        psS = ctx.enter_context(tc.tile_pool(name="psS", bufs=5, space="PSUM"))
        psO = ctx.enter_context(tc.tile_pool(name="psO", bufs=2, space="PSUM"))
        ocpool = ctx.enter_context(tc.tile_pool(name="oc", bufs=2))
        spool = ctx.enter_context(tc.tile_pool(name="sgn", bufs=2))

        # persistent tiles (w DMAs are emitted inside whole_body, per-m
        # chunks on the sync queue, after the first x load)
        wts = [
            const.tile([KP, KC * HID], wdt, tag=f"w{i}", name=f"w{i}")
            for i in range(nws)
        ]
        w2t = const.tile([128, MC * OUT], bf16, tag="w20", name="w20")
        vd = [const.tile([128, MC * BL], f32, tag=f"vd{i}", name=f"vd{i}") for i in range(nvd)]
        nhalf = const.tile([128, 1], f32, tag="nhalf", name="nhalf")
        if RO_HMAT:
            hmt = const.tile([T, T], f32, tag="hs", name="hs")
            octime = const.tile([T, OUT * BL], f32, tag="octime", name="octime")
            vm2 = const.tile([128, NCH], f32, tag="vm2", name="vm2")
            psR = ctx.enter_context(tc.tile_pool(name="psR", bufs=1, space="PSUM"))
        else:
            js = const.tile([OUT, BL], f32, tag="js", name="js")
            vo = const.tile([OUT, BL], f32, tag="vo", name="vo")
            vmax_t = const.tile([OUT, BL], f32, tag="vmax", name="vmax")

        def dma_w():
            for i in range(nws):
                wap = wts[i][:].rearrange("p (k h) -> p k h", k=KC)
                for m in range(MC):
                    nc.sync.dma_start(
                        wap[:, :, m * 128:(m + 1) * 128],
                        wps[i][:, :, m * 128:(m + 1) * 128],
                    )
            nc.sync.dma_start(w2t[:], w2ps[0][:, :])
            if RO_HMAT:
                nc.sync.dma_start(hmt[:], hs_p[:, :])

        def dma_x(g):
            c0, ncol = t0s[g] * BL, tgs[g] * BL
            tiles = []
            for si in range(nxs):
                xt = xpool.tile([KP, KC * tg * BL], xdt, tag=f"x{si}", name=f"x{si}")
                xv = xt[:, :KC * ncol].rearrange("p (k c) -> p k c", k=KC)
                if g <= 1:
                    # per-k chunks, round-robin across idle queues, so the
                    # first matmuls aren't gated on one serial SWDGE stream
                    qs = [nc.gpsimd, nc.scalar]
                    for k in range(KC):
                        qs[k % len(qs)].dma_start(
                            xv[:, k:k + 1, :], xps[si][:, k:k + 1, c0:c0 + ncol]
                        )
                else:
                    nc.gpsimd.dma_start(xv, xps[si][:, :, c0:c0 + ncol])
                tiles.append(xt)
            return tiles

        def fc1(g, xts):
            tgl = tgs[g]
            ncol = tgl * BL
            sw = swpool.tile([128, tg * MC * BL], f32, tag="swin", name="swin")
            for m in range(MC):
                ps = psS.tile([128, tg * BL], f32, tag="psS", name="psS")
                idx, last = 0, len(passes) * KC - 1
                for (xi, wi) in passes:
                    wap = wts[wi][:].rearrange("p (k h) -> p k h", k=KC)
                    xap = xts[xi][:, :KC * ncol].rearrange(
                        "p (k c) -> p k c", k=KC
                    )
                    for k in range(KC):
                        lhsT = wap[:, k, m * 128:(m + 1) * 128]
                        rhs = xap[:, k, :]
                        nc.tensor.matmul(
                            ps[:, :ncol], lhsT, rhs,
                            start=(idx == 0), stop=(idx == last)
                        )
                        idx += 1
                dst = sw[:, :tgl * MC * BL].rearrange(
                    "p (t m b) -> p t m b", t=tgl, m=MC
                )[:, :, m, :]
                nc.scalar.copy(
                    dst, ps[:, :ncol].rearrange("p (t b) -> p t b", t=tgl)
                )
            return sw

        def ro_step(oc, tloc):
            # one LI-readout step (DVE; GPSIMD has no TensorScalarPtr on HW)
            nc.vector.scalar_tensor_tensor(
                vo[:], vo[:], 0.9, js[:], op0=Alu.mult, op1=Alu.add
            )
            nc.vector.tensor_tensor(vmax_t[:], vmax_t[:], vo[:], op=Alu.max)
            nc.vector.scalar_tensor_tensor(
                js[:], js[:], 0.8, oc[:, tloc * BL:(tloc + 1) * BL],
                op0=Alu.mult, op1=Alu.add,
            )

        def scan(g, sw, ro):
            # sg is (m t b) so fc_out's rhs slices are contiguous 2-D
            tgl = tgs[g]
            sg = sgpool.tile([128, tg * MC * BL], bf16, tag="sgn", name="sgn")
            sgap = sg[:, :tgl * MC * BL].rearrange(
                "p (m t b) -> p m t b", m=MC, t=tgl
            )
            w = MC * BL
            if level < 2:
                return sg
            for tloc in range(tgl):
                t = t0s[g] + tloc
                a = vd[t % nvd]
                b = vd[(t + 1) % nvd]
                vr = vd[(t + 2) % nvd]
                # DVE: the serial v-chain, back-to-back on one engine
                nc.vector.scalar_tensor_tensor(
                    vr[:], a[:], 0.5, a[:], op0=Alu.is_le, op1=Alu.mult
                )
                nc.vector.scalar_tensor_tensor(
                    b[:], vr[:], 0.9, sw[:, tloc * w:(tloc + 1) * w],
                    op0=Alu.mult, op1=Alu.add,
                )
                if level >= 3:
                    # spike values, off the serial chain: ACT computes
                    # sign(v-0.5) in bf16, DVE finishes z=max(s,0) at
                    # 2-byte (2x) rate
                    st = spool.tile([128, MC * BL], bf16, tag="st", name="st")
                    nc.scalar.activation(st[:], a[:], Act.Sign, bias=nhalf[:])
                    nc.vector.tensor_scalar(
                        sgap[:, :, tloc, :], st[:].rearrange(
                            "p (m b) -> p m b", m=MC
                        ), 0.0, None, op0=Alu.max
                    )
                if ro and level >= 5:
                    ro_step(*ro.pop(0))
                    if len(ro) > tgl - 1 - tloc:  # backlog: drain 2 per step
                        ro_step(*ro.pop(0))
            return sg

        def fc_out(g, sg):
            # Pool can't read PSUM on real HW, so oc hops through SBUF (ACT)
            tgl = tgs[g]
            po = psO.tile([OUT, tg * BL], f32, tag="psO", name="psO")
            oc = ocpool.tile([OUT, tg * BL], f32, tag="ocs", name="ocs")
            if level < 4:
                return oc
            for m in range(MC):
                rhs = sg[:, m * tgl * BL:(m + 1) * tgl * BL]
                lhsT = w2t[:, m * OUT:(m + 1) * OUT]
                nc.tensor.matmul(
                    po[:, :tgl * BL], lhsT, rhs,
                    start=(m == 0), stop=(m == MC - 1)
                )
            nc.scalar.copy(oc[:, :tgl * BL], po[:, :tgl * BL])
            if RO_HMAT:
                # transpose oc into octime [t, (o b)] via per-t SBUF DMAs
                ocv = oc[:, :tgl * BL].rearrange("o (t b) -> o t b", t=tgl)
                for tloc in range(tgl):
                    t = t0s[g] + tloc
                    nc.sync.dma_start(
                        octime[t:t + 1, :].rearrange(
                            "t (o b) -> t o b", o=OUT
                        ),
                        ocv[:, tloc, :],
                    )
            return oc

        def whole_body():
            # init state, then software-pipelined emission: fc1 leads the
            # scan by 1 group; fc_out of g-2 is emitted before scan(g-1),
            # whose Pool loop carries the interleaved readout of g-2.
            xts = dma_x(0)
            dma_w()
            nc.vector.memset(vd[0][:], 0.0)
            nc.vector.memset(nhalf[:], -0.5)
            if not RO_HMAT:
                nc.vector.memset(js[:], 0.0)
                nc.vector.memset(vo[:], 0.0)
                nc.vector.memset(vmax_t[:], 0.0)
            sws, sgs = {}, {}
            ro = []
            sws[0] = fc1(0, xts)
            for g in range(1, ng):
                xts = dma_x(g)
                sws[g] = fc1(g, xts)
                if g >= 2:
                    po = fc_out(g - 2, sgs.pop(g - 2))
                    if not RO_HMAT:
                        ro.extend((po, i) for i in range(tgs[g - 2]))
                sgs[g - 1] = scan(g - 1, sws.pop(g - 1), ro)
            po = fc_out(ng - 2, sgs.pop(ng - 2))
            if not RO_HMAT:
                ro.extend((po, i) for i in range(tgs[ng - 2]))
            sgs[ng - 1] = scan(ng - 1, sws.pop(ng - 1), ro)
            po = fc_out(ng - 1, sgs.pop(ng - 1))
            if not RO_HMAT:
                ro.extend((po, i) for i in range(tgs[ng - 1]))
            if RO_HMAT:
                # vo[t'] for all t' in one small matmul per 128-column chunk
                # of the transposed oc, then max over t' on DVE
                for c in range(NCH):
                    pr = psR.tile([128, T], f32, tag="psR", name="psR")
                    nc.tensor.matmul(
                        pr[:], octime[:, c * 128:(c + 1) * 128], hmt[:],
                        start=True, stop=True,
                    )
                    nc.vector.tensor_reduce(
                        vm2[:, c:c + 1], pr[:],
                        axis=mybir.AxisListType.X, op=Alu.max,
                    )
                nc.sync.dma_start(vmax_p[:, :], vm2[:])
            else:
                while ro:  # flush in order (js/vo are sequential state)
                    ro_step(*ro.pop(0))
                nc.sync.dma_start(vmax_p[:, :], vmax_t[:])

        if repeat > 1:
            with tc.For_i(0, repeat, 1):
                whole_body()
        else:
            whole_body()

    nc.compile()
    return nc


def _prep_inputs(x, w1, w_out, mode):
    x = np.ascontiguousarray(x, np.float32).reshape(T, B, IN)
    # i1[t] = 0.8*i1[t-1] + x[t] @ w1T  ==  prefilter(x)[t] @ w1T
    xf = np.empty_like(x)
    acc = np.zeros((B, IN), np.float32)
    e8 = np.float32(0.8)
    for t in range(T):
        acc = e8 * acc + x[t]
        xf[t] = acc
    xs = np.float32(0.1) * xf                       # S = xs @ w1T
    w1T = np.ascontiguousarray(w1, np.float32).T    # [IN, HID]

    if mode == "r1":
        xparts = [_round_mant(xs, 11)]
        wparts = [_layout_w(_round_mant(w1T, 11))]
    else:
        xh = _to_bf16(xs)
        xl = _to_bf16(xs - xh.astype(np.float32))
        xparts = [xh, xl]
        wh = _to_bf16(w1T)
        wl = _to_bf16(w1T - wh.astype(np.float32))
        wparts = [_layout_w(wh), _layout_w(wl)]

    w2 = np.float32(0.1) * np.ascontiguousarray(w_out, np.float32).T  # [HID,OUT]

    common = {
        "w20": _layout_w2(_to_bf16(w2)),
    }
    if RO_HMAT:
        # hs[s, t'] = coefficient of oc[s] in vo-after-step-t' (impulse sim
        # of: vo <- 0.9*vo + js; record; js <- 0.8*js + oc[t])
        H = np.zeros((T, T), np.float64)
        js_c = np.zeros(T, np.float64)
        vo_c = np.zeros(T, np.float64)
        for t in range(T):
            vo_c = 0.9 * vo_c + js_c
            H[t] = vo_c
            js_c = 0.8 * js_c
            js_c[t] += 1.0
        common["hs"] = np.ascontiguousarray(H.T, np.float32)
    for i, wp in enumerate(wparts):
        common[f"w{i}"] = wp

    in_maps = []
    for c in range(NCORES):
        m = dict(common)
        for i, xp in enumerate(xparts):
            xc = xp[:, c * BL:(c + 1) * BL, :].reshape(TB, IN)
            m[f"x{i}"] = _layout_x(xc)
        in_maps.append(m)
    return in_maps


def kernel(x, w1, w_out):
    global LAST_RESULT
    from concourse.bass_utils import run_bass_kernel_spmd

    if MODE not in _CACHE:
        _CACHE[MODE] = _build_nc(MODE, tg=TG)
    nc = _CACHE[MODE]
    in_maps = _prep_inputs(np.asarray(x), np.asarray(w1), np.asarray(w_out), MODE)
    res = run_bass_kernel_spmd(nc, in_maps, list(range(NCORES)), trace=TRACE)
    LAST_RESULT = res
    out = np.empty((B, OUT), np.float32)
    for c in range(NCORES):
        vm = np.asarray(res.results[c]["vmax"])
        if RO_HMAT:
            # [128, NCH] -> flat (o*BL + b) -> [BL, OUT]
            out[c * BL:(c + 1) * BL, :] = vm.T.reshape(OUT * BL).reshape(
                OUT, BL
            ).T
        else:
            out[c * BL:(c + 1) * BL, :] = vm.T
    return out
